# revision 13
# baseline (speedup 1.0000x reference)
# Trainium2 Bass kernel for CapsNet dynamic routing (nn_DigiCap).
#
#   u_hat = einsum('bid,ikdo->biko', x, W);  3 routing iterations of
#   softmax / weighted-sum / squash over K=32 output capsules.
#
# Strategy: shard the 2048 input capsules (i) across 8 cores (I_loc=256).
# All tensors bf16 on the wire and in SBUF; f32 only in PSUM + small
# squash/collective math.
#
# Per core:
#   Phase A: u_hat via 2-capsule block-diagonal matmuls: lhsT =
#     x2blk [32=(jj,d), 128=(j',b)] (2x2 block-diag of [16d x 64b]),
#     rhs = W-pair [32, 512=(k,o)], out PSUM [128=(j',b), 512].
#     A twin matmul accumulates s0 = sum_i u_hat into a per-q PSUM bank.
#     The v0 chain (fold + bf16 AllReduce) is issued right after the s0
#     matmuls so it overlaps the u_hat matmuls; squash/G-update are
#     emitted mid-way through the u_hat loop (after the collective is
#     done) so routing can start immediately when phase A ends.
#     PSUM->SBUF evac (f32->bf16, (k,o)->(o,k)) split across ACT/DVE.
#   Routing passes t=1,2 on DVE (bf16 2x mode): b = sum_o u*G via
#     mul+tree, softmax_k, s = sum_i c*u via mul+tree. exp() runs on
#     ACT per-chunk, pipelined with the next b-chunk on DVE.
#   s partial folded across partition halves via a cross-partition DVE
#   add, AllReduced in bf16 across cores, squash recomputed per core.
#   sqrt() inside squash is computed as exp(0.5*ln(x)) so the single
#   ACT table set natural_log_exp_and_others serves softmax + squash
#   (no per-iteration table thrash).
#
# SBUF u layout: [128 part=(j',b), i_lo=128, o=16, k=32] bf16 where the
# local capsule index is i = 2*i_lo + j'.

import numpy as np

B, I_TOT, D, K, O = 64, 2048, 16, 32, 16
NCORES = 8
I_LOC = I_TOT // NCORES     # 256
NG8 = 32                    # groups of 8 capsules (DMA/matmul bursts)
IC = 32                     # i_lo chunk for routing passes

_CACHE = {}


def _build_bass():
    import concourse.bacc as bacc
    import concourse.tile as tile
    from concourse import mybir

    f32 = mybir.dt.float32
    bf16 = mybir.dt.bfloat16
    Alu = mybir.AluOpType
    Act = mybir.ActivationFunctionType
    X = mybir.AxisListType.X

    nc = bacc.Bacc("TRN2", target_bir_lowering=False, debug=False,
                   num_devices=NCORES)

    # x2blk[p=32q+16jj+d, g8, m=64j'+b] = x[b, 8g8+2q+jj, d] * (jj==j')
    x2_d = nc.dram_tensor("x2", [128, NG8, 128], bf16, kind="ExternalInput")
    # x8e[p=16j+d, g8, m=64e+b] = x[b, 8g8+j, d] * ((j&1)==e)  (s0 pass)
    x8_d = nc.dram_tensor("x8e", [128, NG8, 128], bf16, kind="ExternalInput")
    # Wt[g8, 16j+d, 16k+o] = W[8g8+j, k, d, o]
    W_d = nc.dram_tensor("Wt", [NG8, 128, 512], bf16, kind="ExternalInput")
    v_d = nc.dram_tensor("v_out", [64, 512], f32, kind="ExternalOutput")

    with tile.TileContext(nc) as tc:
        with (
            tc.tile_pool(name="big", bufs=1) as big,
            tc.tile_pool(name="psum", bufs=1, space="PSUM") as psp,
            tc.tile_pool(name="dram", bufs=1, space="DRAM") as dp,
            tc.tile_pool(name="small", bufs=1) as sp,
            tc.tile_pool(name="work", bufs=1) as wk,
        ):
            # u split in two tiles so the t=1 b-pass (reading u_a only)
            # isn't blocked by whole-tile dependencies on the last evacs
            u_a = big.tile([128, 96, 16, 32], bf16)      # 96 KB/part
            u_b = big.tile([128, 32, 16, 32], bf16)      # 32 KB/part
            bA = big.tile([128, 128, 32], bf16)          # 8 KB/part

            ps0 = psp.tile([128, 512], f32)              # bank 0: s0 accum

            # ---------------- shared collective/squash helpers ----------
            bounce = dp.tile([64, 512], bf16)
            bounce2 = dp.tile([64, 512], bf16)
            G = wk.tile([64, 16, 32], f32, tag="G")
            vt = wk.tile([64, 16, 32], f32, tag="vt")
            n2s = wk.tile([64, 32], f32, tag="n2")
            sqs = wk.tile([64, 32], f32, tag="sq")
            Grep = sp.tile([128, 16, 32], bf16)

            def fold_and_allreduce(s_tile, s_full):
                # fold j' halves on DVE (TENSOR_TENSOR needs equal base
                # partitions, so move the high half down with a copy
                # first), bounce through DRAM for the bf16 AllReduce.
                s_hi = wk.tile([64, 16, 32], f32, tag="s_hi")
                nc.vector.tensor_copy(s_hi[:], s_tile[64:128])
                s_fb = wk.tile([64, 16, 32], bf16, tag="s_fb")
                nc.vector.tensor_add(s_fb[:], s_tile[0:64], s_hi[:])
                nc.gpsimd.dma_start(
                    bounce[:], s_fb[:].rearrange("p o k -> p (o k)"))
                nc.gpsimd.collective_compute(
                    "AllReduce", Alu.add,
                    replica_groups=[list(range(NCORES))],
                    ins=[bounce.opt()], outs=[bounce2.opt()])
                nc.gpsimd.dma_start(
                    s_full[:].rearrange("p o k -> p (o k)"), bounce2[:])

            def squash(s_full, vout):
                # vout = s * sqrt(n2)/(1+n2),  n2 = sum_o s^2  per (b,k)
                # the sqrt table set is prefetched by a dummy Sqrt during
                # the s-pass, so no ACT table load lands on this path
                ss = wk.tile([64, 16, 32], f32, tag="ss")
                nc.vector.tensor_mul(ss[:], s_full[:], s_full[:])
                n2 = n2s
                nc.vector.tensor_reduce(
                    n2[:], ss[:].rearrange("p o k -> p k o"), axis=X,
                    op=Alu.add)
                sq = sqs
                nc.scalar.activation(sq[:], n2[:], Act.Sqrt)
                den = wk.tile([64, 32], f32, tag="den")
                nc.vector.tensor_scalar_add(den[:], n2[:], 1.0)
                rec = wk.tile([64, 32], f32, tag="rec")
                nc.vector.reciprocal(rec[:], den[:])
                scl = wk.tile([64, 32], f32, tag="scl")
                nc.vector.tensor_mul(scl[:], sq[:], rec[:])
                sclb = scl[:].unsqueeze(1).broadcast_to([64, 16, 32])
                nc.vector.tensor_mul(vout[:], s_full[:], sclb)

            def update_g(first):
                # Grep first (the b-pass only needs Grep; the f32 G update
                # happens off the critical path)
                if first:
                    nc.vector.tensor_copy(Grep[0:64], vt[:])
                    nc.vector.tensor_copy(Grep[64:128], Grep[0:64])
                    nc.vector.tensor_copy(G[:], vt[:])
                else:
                    nc.vector.tensor_add(Grep[0:64], G[:], vt[:])
                    nc.vector.tensor_copy(Grep[64:128], Grep[0:64])
                    nc.vector.tensor_add(G[:], G[:], vt[:])

            s_p = wk.tile([128, 16, 32], f32, tag="s_p")
            s_full0 = wk.tile([64, 16, 32], bf16, tag="s_full0")
            s_full = wk.tile([64, 16, 32], bf16, tag="s_full")

            # ---------------- Phase A: s0 (early), then u_hat ----------
            with (
                tc.tile_pool(name="pha", bufs=1) as pha,
                tc.tile_pool(name="wp", bufs=6) as wp,
                tc.tile_pool(name="wp2", bufs=4) as wp2,
                tc.tile_pool(name="pup", bufs=3, space="PSUM") as pup,
            ):
                x2 = pha.tile([128, NG8, 128], bf16)
                x8 = pha.tile([128, NG8, 128], bf16)
                for c4 in range(4):
                    nc.sync.dma_start(
                        x8[:, 8 * c4:8 * c4 + 8, :], x8_d[:, 8 * c4:8 * c4 + 8, :])
                nc.sync.dma_start(x2[:], x2_d.ap())
                # s0 pass first: dense [128]-contract accumulation, so the
                # AllReduce + squash for v0 overlap the u_hat matmuls.
                # ws DMAs ride the gpsimd queue so they don't pace behind
                # the u_hat W stream on the sync queue -- back-to-back
                # matmuls also keep the PE HAM un-throttled (2.4 GHz).
                with tc.high_priority():
                    for g8 in range(NG8):
                        ws = wp.tile([128, 512], bf16, tag="ws")
                        nc.gpsimd.dma_start(ws[:], W_d[g8])
                        nc.tensor.matmul(
                            ps0[:], x8[:, g8, :], ws[:],
                            start=(g8 == 0), stop=(g8 == NG8 - 1))
                # v0 collective chain, issued now so it overlaps u_hat.
                # s0 partial (both j'-parity halves on partitions): * 1/K
                s_pf = s_p[:].rearrange("p o k -> p (o k)")
                nc.vector.tensor_scalar_mul(s_pf, ps0[:], 1.0 / K)
                # note: s_p free layout is (k,o) order from PSUM here; the
                # fold/allreduce treat it as an opaque 512 vec and the v0
                # squash relabels via s0v below.
                fold_and_allreduce(s_p, s_full0)

                for g8 in range(NG8):
                    w = wp2.tile([128, 512], bf16, tag="wburst")
                    nc.sync.dma_start(w[:], W_d[g8])
                    pu = None
                    for q in range(4):
                        if q % 2 == 0:
                            pu = pup.tile([128, 2, 512], f32, tag="pu")
                        lhsT = x2[32 * q:32 * q + 32, g8, :]   # [32, 128]
                        rhs = w[32 * q:32 * q + 32, :]         # [32, 512]
                        nc.tensor.matmul(
                            pu[:, q % 2, :], lhsT, rhs,
                            start=True, stop=True,
                            tile_position=(32 * q, 0))
                        if q % 2 == 1:
                            # evacuate 2 groups: psum (k o) -> u (o k)
                            g = 4 * g8 + q - 1
                            src = pu[:].rearrange(
                                "p r (k o) -> p r o k", k=K)
                            dst = (u_a[:, g:g + 2] if g < 96
                                   else u_b[:, g - 96:g - 96 + 2])
                            # Pool/DMA can't read PSUM: alternate ACT/DVE
                            # until g8 24, then ACT-only -- frees the DVE
                            # queue to start the t=1 b-pass ~15us before
                            # phase A fully drains
                            if g8 >= 24 or (2 * g8 + q // 2) % 2 == 0:
                                nc.scalar.copy(dst, src)
                            else:
                                nc.vector.tensor_copy(dst, src)
                    if g8 == 20:
                        # collective is done by now; emit v0 squash +
                        # G-update here so ACT/DVE interleave it with the
                        # remaining evacs instead of stalling afterwards.
                        # s_full0 content is (k,o); relabel to (o,k).
                        s0v = wk.tile([64, 16, 32], f32, tag="s0v")
                        nc.vector.tensor_copy(
                            s0v[:],
                            s_full0[:].rearrange("p o k -> p (o k)")
                            .rearrange("p (k o) -> p o k", k=K))
                        squash(s0v, vt)
                        update_g(first=True)

            # ---------------- routing passes ----------------
            # NOTE: DVE and GpSimd share SBUF ports -- running Pool TT
            # concurrently with DVE TT slows DVE ~1.5x (measured). All
            # routing TT stays on DVE.
            # Routing-only tiles live in their own pool (rt) so their
            # 39KB/part doesn't shrink phase A's W pools.
            rt_cm = tc.tile_pool(name="rt", bufs=1)
            rt = rt_cm.__enter__()
            # (utile, utile-local i0, bA i0)
            DVE_CH = [(u_a, 0, 0), (u_a, IC, IC), (u_a, 2 * IC, 2 * IC),
                      (u_b, 0, 3 * IC)]
            tmp = rt.tile([128, IC, 16, 32], bf16, tag="tmp")
            tl = rt.tile([128, 16, 32], f32, tag="tl")
            zt = rt.tile([128, 128, 16], bf16, tag="zt")
            rz = rt.tile([128, 128], bf16, tag="rz")
            rzf = rt.tile([128, 128], f32, tag="rzf")
            sq_pre = rt.tile([64, 32], f32, tag="sq_pre")
            zro = rt.tile([64, 32], f32, tag="zro")

            def b_chunk(eng, tm, ut, ui0, i0, ln):
                gb = Grep[:].unsqueeze(1).broadcast_to([128, ln, 16, 32])
                tc_ = tm[:, 0:ln]
                eng.tensor_mul(tc_, ut[:, ui0:ui0 + ln], gb)
                eng.tensor_add(
                    tc_[:, :, 0:8], tc_[:, :, 0:8], tc_[:, :, 8:16])
                eng.tensor_add(
                    tc_[:, :, 0:4], tc_[:, :, 0:4], tc_[:, :, 4:8])
                eng.tensor_add(
                    tc_[:, :, 0:2], tc_[:, :, 0:2], tc_[:, :, 2:4])
                eng.tensor_add(
                    bA[:, i0:i0 + ln, :], tc_[:, :, 0], tc_[:, :, 1])

            def s_chunk(eng, tm, ut, ui0, i0, ln, acc, tacc, first):
                cb = bA[:, i0:i0 + ln, :].unsqueeze(2).broadcast_to(
                    [128, ln, 16, 32])
                tc_ = tm[:, 0:ln]
                eng.tensor_mul(tc_, ut[:, ui0:ui0 + ln], cb)
                h = ln // 2
                while h >= 2:
                    eng.tensor_add(
                        tc_[:, 0:h], tc_[:, 0:h], tc_[:, h:2 * h])
                    h //= 2
                if first:
                    eng.tensor_add(acc[:], tc_[:, 0], tc_[:, 1])
                else:
                    eng.tensor_add(tacc[:], tc_[:, 0], tc_[:, 1])
                    eng.tensor_add(acc[:], acc[:], tacc[:])

            for t in (1, 2):
                # b = sum_o u * G; exp per chunk on ACT overlaps the next
                # chunk's DVE work
                for ut, ui0, i0 in DVE_CH:
                    b_chunk(nc.vector, tmp, ut, ui0, i0, IC)
                    nc.scalar.activation(
                        bA[:, i0:i0 + IC, :], bA[:, i0:i0 + IC, :], Act.Exp)
                # dummy Sqrt prefetches the sqrt table set while ACT is
                # idle (the real squash Sqrt then loads nothing); chained
                # into a x0.0 add below so dead-code elimination keeps it
                nc.scalar.activation(sq_pre[:], n2s[:], Act.Sqrt)
                nc.vector.tensor_scalar_mul(zro[:], sq_pre[:], 0.0)
                # c = softmax_k(b)   (no max subtraction; |b| is small)
                nc.vector.tensor_add(
                    zt[:], bA[:, :, 0:16], bA[:, :, 16:32])
                nc.vector.tensor_add(
                    zt[:, :, 0:8], zt[:, :, 0:8], zt[:, :, 8:16])
                nc.vector.tensor_add(
                    zt[:, :, 0:4], zt[:, :, 0:4], zt[:, :, 4:8])
                nc.vector.tensor_add(
                    zt[:, :, 0:2], zt[:, :, 0:2], zt[:, :, 2:4])
                nc.vector.tensor_add(
                    zt[:, :, 0], zt[:, :, 0], zt[:, :, 1])
                nc.vector.reciprocal(rzf[:], zt[:, :, 0])
                nc.vector.tensor_copy(rz[:], rzf[:])
                rzb = rz[:].unsqueeze(2).broadcast_to([128, 128, 32])
                # c overwrites bA in place
                nc.vector.tensor_mul(bA[:], bA[:], rzb)
                # s(partial) = sum_i c * u
                for n, (ut, ui0, i0) in enumerate(DVE_CH):
                    s_chunk(nc.vector, tmp, ut, ui0, i0, IC, s_p, tl, n == 0)
                fold_and_allreduce(s_p, s_full)
                squash(s_full, vt)
                if t == 1:
                    update_g(first=False)
                    nc.vector.tensor_add(G[:, 0, :], G[:, 0, :], zro[:])
                else:
                    nc.vector.tensor_add(vt[:, 0, :], vt[:, 0, :], zro[:])

            # write out v2 (free layout o*32+k; host reorders)
            nc.sync.dma_start(
                v_d.ap(), vt[:].rearrange("p o k -> p (o k)"))
            rt_cm.__exit__(None, None, None)

    nc.compile()
    return nc


def _prep_inputs(x, W):
    """Host-side shard + relayout + bf16 cast. Per-core input maps."""
    import ml_dtypes

    bf16 = ml_dtypes.bfloat16
    in_maps = []
    for c in range(NCORES):
        lo = c * I_LOC
        xc = x[:, lo:lo + I_LOC, :]                      # [64, 256, 16]
        # xr[i, d, b]
        xr = np.ascontiguousarray(xc.transpose(1, 2, 0))     # [256, 16, 64]
        # x2[32q+16jj+d, g8, 64j'+b] = xr[8g8+2q+jj, d, b] * (jj==j')
        xg = xr.reshape(NG8, 4, 2, D, B)                 # [g8, q, jj, d, b]
        x2 = np.zeros((NG8, 4, 2, D, 2, B), dtype=np.float32)
        x2[:, :, 0, :, 0, :] = xg[:, :, 0]
        x2[:, :, 1, :, 1, :] = xg[:, :, 1]
        # -> [ (q, jj, d) = 128, g8, (j', b) = 128 ]
        x2 = np.ascontiguousarray(
            x2.transpose(1, 2, 3, 0, 4, 5).reshape(128, NG8, 128)
        ).astype(bf16)
        # x8e[16j+d, g8, 64e+b] = xr[8g8+j, d, b] * ((j&1)==e)
        x8g = xr.reshape(NG8, 8, D, B)                   # [g8, j, d, b]
        x8 = np.zeros((NG8, 8, D, 2, B), dtype=np.float32)
        x8[:, 0::2, :, 0, :] = x8g[:, 0::2]
        x8[:, 1::2, :, 1, :] = x8g[:, 1::2]
        x8 = np.ascontiguousarray(
            x8.transpose(1, 2, 0, 3, 4).reshape(128, NG8, 128)).astype(bf16)
        # Wt[g8, 16j+d, 16k+o] = W[lo+8g8+j, k, d, o]
        Wc = W[lo:lo + I_LOC]                            # [256, 32, 16, 16]
        Wt = np.ascontiguousarray(
            Wc.reshape(NG8, 8, K, D, O).transpose(0, 1, 3, 2, 4)
            .reshape(NG8, 128, K * O)).astype(bf16)
        in_maps.append({"x2": x2, "x8e": x8, "Wt": Wt})
    return in_maps


def kernel(**inputs):
    from concourse.bass_utils import run_bass_kernel_spmd

    x = np.ascontiguousarray(inputs["inputs"], dtype=np.float32)
    W = np.ascontiguousarray(inputs["W"], dtype=np.float32)

    if "nc" not in _CACHE:
        _CACHE["nc"] = _build_bass()
    nc = _CACHE["nc"]

    in_maps = _prep_inputs(x, W)
    res = run_bass_kernel_spmd(nc, in_maps, core_ids=list(range(NCORES)))
    v = res.results[0]["v_out"]                          # [64, 512] (o,k)
    return np.ascontiguousarray(
        v.reshape(B, O, K).transpose(0, 2, 1)).astype(np.float32)


# revision 14
# speedup vs baseline: 1.0257x; 1.0257x over previous
# Trainium2 Bass kernel for CapsNet dynamic routing (nn_DigiCap).
#
#   u_hat = einsum('bid,ikdo->biko', x, W);  3 routing iterations of
#   softmax / weighted-sum / squash over K=32 output capsules.
#
# Strategy: shard the 2048 input capsules (i) across 8 cores (I_loc=256).
# All tensors bf16 on the wire and in SBUF; f32 only in PSUM + small
# squash/collective math.
#
# Per core:
#   Phase A: u_hat via 2-capsule block-diagonal matmuls: lhsT =
#     x2blk [32=(jj,d), 128=(j',b)] (2x2 block-diag of [16d x 64b]),
#     rhs = W-pair [32, 512=(k,o)], out PSUM [128=(j',b), 512].
#     A twin matmul accumulates s0 = sum_i u_hat into a per-q PSUM bank.
#     The v0 chain (fold + bf16 AllReduce) is issued right after the s0
#     matmuls so it overlaps the u_hat matmuls; squash/G-update are
#     emitted mid-way through the u_hat loop (after the collective is
#     done) so routing can start immediately when phase A ends.
#     PSUM->SBUF evac (f32->bf16, (k,o)->(o,k)) split across ACT/DVE.
#   Routing passes t=1,2 on DVE (bf16 2x mode): b = sum_o u*G via
#     mul+tree, softmax_k, s = sum_i c*u via mul+tree. exp() runs on
#     ACT per-chunk, pipelined with the next b-chunk on DVE.
#   s partial folded across partition halves via a cross-partition DVE
#   add, AllReduced in bf16 across cores, squash recomputed per core.
#   sqrt() inside squash is computed as exp(0.5*ln(x)) so the single
#   ACT table set natural_log_exp_and_others serves softmax + squash
#   (no per-iteration table thrash).
#
# SBUF u layout: [128 part=(j',b), i_lo=128, o=16, k=32] bf16 where the
# local capsule index is i = 2*i_lo + j'.

import numpy as np

B, I_TOT, D, K, O = 64, 2048, 16, 32, 16
NCORES = 8
I_LOC = I_TOT // NCORES     # 256
NG8 = 32                    # groups of 8 capsules (DMA/matmul bursts)
IC = 32                     # i_lo chunk for routing passes

_CACHE = {}


def _build_bass():
    import concourse.bacc as bacc
    import concourse.tile as tile
    from concourse import mybir

    f32 = mybir.dt.float32
    bf16 = mybir.dt.bfloat16
    Alu = mybir.AluOpType
    Act = mybir.ActivationFunctionType
    X = mybir.AxisListType.X

    nc = bacc.Bacc("TRN2", target_bir_lowering=False, debug=False,
                   num_devices=NCORES)

    # x2blk[p=32q+16jj+d, g8, m=64j'+b] = x[b, 8g8+2q+jj, d] * (jj==j')
    x2_d = nc.dram_tensor("x2", [128, NG8, 128], bf16, kind="ExternalInput")
    # x8e[p=16j+d, g8, m=64e+b] = x[b, 8g8+j, d] * ((j&1)==e)  (s0 pass)
    x8_d = nc.dram_tensor("x8e", [128, NG8, 128], bf16, kind="ExternalInput")
    # Wt[g8, 16j+d, 16k+o] = W[8g8+j, k, d, o]
    W_d = nc.dram_tensor("Wt", [NG8, 128, 512], bf16, kind="ExternalInput")
    v_d = nc.dram_tensor("v_out", [64, 512], f32, kind="ExternalOutput")

    with tile.TileContext(nc) as tc:
        with (
            tc.tile_pool(name="big", bufs=1) as big,
            tc.tile_pool(name="psum", bufs=1, space="PSUM") as psp,
            tc.tile_pool(name="dram", bufs=1, space="DRAM") as dp,
            tc.tile_pool(name="small", bufs=1) as sp,
            tc.tile_pool(name="work", bufs=1) as wk,
        ):
            u = big.tile([128, 128, 16, 32], bf16)       # 128 KB/part
            bA = big.tile([128, 128, 32], bf16)          # 8 KB/part

            ps0 = psp.tile([128, 512], f32)              # bank 0: s0 accum

            # ---------------- shared collective/squash helpers ----------
            bounce = dp.tile([64, 512], bf16)
            bounce2 = dp.tile([64, 512], bf16)
            G = wk.tile([64, 16, 32], f32, tag="G")
            vt = wk.tile([64, 16, 32], f32, tag="vt")
            n2s = wk.tile([64, 32], f32, tag="n2")
            sqs = wk.tile([64, 32], f32, tag="sq")
            Grep = sp.tile([128, 16, 32], bf16)

            def fold_and_allreduce(s_tile, s_full):
                # fold j' halves on DVE (TENSOR_TENSOR needs equal base
                # partitions, so move the high half down with a copy
                # first), bounce through DRAM for the bf16 AllReduce.
                s_hi = wk.tile([64, 16, 32], f32, tag="s_hi")
                nc.vector.tensor_copy(s_hi[:], s_tile[64:128])
                s_fb = wk.tile([64, 16, 32], bf16, tag="s_fb")
                nc.vector.tensor_add(s_fb[:], s_tile[0:64], s_hi[:])
                nc.gpsimd.dma_start(
                    bounce[:], s_fb[:].rearrange("p o k -> p (o k)"))
                nc.gpsimd.collective_compute(
                    "AllReduce", Alu.add,
                    replica_groups=[list(range(NCORES))],
                    ins=[bounce.opt()], outs=[bounce2.opt()])
                nc.gpsimd.dma_start(
                    s_full[:].rearrange("p o k -> p (o k)"), bounce2[:])

            def squash(s_full, vout):
                # vout = s * sqrt(n2)/(1+n2),  n2 = sum_o s^2  per (b,k)
                # the sqrt table set is prefetched by a dummy Sqrt during
                # the s-pass, so no ACT table load lands on this path
                ss = wk.tile([64, 16, 32], f32, tag="ss")
                nc.vector.tensor_mul(ss[:], s_full[:], s_full[:])
                n2 = n2s
                nc.vector.tensor_reduce(
                    n2[:], ss[:].rearrange("p o k -> p k o"), axis=X,
                    op=Alu.add)
                sq = sqs
                nc.scalar.activation(sq[:], n2[:], Act.Sqrt)
                den = wk.tile([64, 32], f32, tag="den")
                nc.vector.tensor_scalar_add(den[:], n2[:], 1.0)
                rec = wk.tile([64, 32], f32, tag="rec")
                nc.vector.reciprocal(rec[:], den[:])
                scl = wk.tile([64, 32], f32, tag="scl")
                nc.vector.tensor_mul(scl[:], sq[:], rec[:])
                sclb = scl[:].unsqueeze(1).broadcast_to([64, 16, 32])
                nc.vector.tensor_mul(vout[:], s_full[:], sclb)

            def update_g(first):
                # Grep first (the b-pass only needs Grep; the f32 G update
                # happens off the critical path)
                if first:
                    nc.vector.tensor_copy(Grep[0:64], vt[:])
                    nc.vector.tensor_copy(Grep[64:128], Grep[0:64])
                    nc.vector.tensor_copy(G[:], vt[:])
                else:
                    nc.vector.tensor_add(Grep[0:64], G[:], vt[:])
                    nc.vector.tensor_copy(Grep[64:128], Grep[0:64])
                    nc.vector.tensor_add(G[:], G[:], vt[:])

            s_p = wk.tile([128, 16, 32], f32, tag="s_p")
            s_full0 = wk.tile([64, 16, 32], bf16, tag="s_full0")
            s_full = wk.tile([64, 16, 32], bf16, tag="s_full")

            # ---------------- Phase A: s0 (early), then u_hat ----------
            with (
                tc.tile_pool(name="pha", bufs=1) as pha,
                tc.tile_pool(name="wp", bufs=6) as wp,
                tc.tile_pool(name="wp2", bufs=4) as wp2,
                tc.tile_pool(name="pup", bufs=3, space="PSUM") as pup,
            ):
                x2 = pha.tile([128, NG8, 128], bf16)
                x8 = pha.tile([128, NG8, 128], bf16)
                for c4 in range(4):
                    nc.sync.dma_start(
                        x8[:, 8 * c4:8 * c4 + 8, :], x8_d[:, 8 * c4:8 * c4 + 8, :])
                nc.sync.dma_start(x2[:], x2_d.ap())
                # s0 pass first: dense [128]-contract accumulation, so the
                # AllReduce + squash for v0 overlap the u_hat matmuls.
                # ws DMAs ride the gpsimd queue so they don't pace behind
                # the u_hat W stream on the sync queue -- back-to-back
                # matmuls also keep the PE HAM un-throttled (2.4 GHz).
                with tc.high_priority():
                    for g8 in range(NG8):
                        ws = wp.tile([128, 512], bf16, tag="ws")
                        nc.gpsimd.dma_start(ws[:], W_d[g8])
                        nc.tensor.matmul(
                            ps0[:], x8[:, g8, :], ws[:],
                            start=(g8 == 0), stop=(g8 == NG8 - 1))
                # v0 collective chain, issued now so it overlaps u_hat.
                # s0 partial (both j'-parity halves on partitions): * 1/K
                s_pf = s_p[:].rearrange("p o k -> p (o k)")
                nc.vector.tensor_scalar_mul(s_pf, ps0[:], 1.0 / K)
                # note: s_p free layout is (k,o) order from PSUM here; the
                # fold/allreduce treat it as an opaque 512 vec and the v0
                # squash relabels via s0v below.
                fold_and_allreduce(s_p, s_full0)

                for g8 in range(NG8):
                    w = wp2.tile([128, 512], bf16, tag="wburst")
                    nc.sync.dma_start(w[:], W_d[g8])
                    pu = None
                    for q in range(4):
                        if q % 2 == 0:
                            pu = pup.tile([128, 2, 512], f32, tag="pu")
                        lhsT = x2[32 * q:32 * q + 32, g8, :]   # [32, 128]
                        rhs = w[32 * q:32 * q + 32, :]         # [32, 512]
                        nc.tensor.matmul(
                            pu[:, q % 2, :], lhsT, rhs,
                            start=True, stop=True,
                            tile_position=(32 * q, 0))
                        if q % 2 == 1:
                            # evacuate 2 groups: psum (k o) -> u (o k)
                            g = 4 * g8 + q - 1
                            src = pu[:].rearrange(
                                "p r (k o) -> p r o k", k=K)
                            dst = u[:, g:g + 2]
                            # Pool/DMA can't read PSUM: alternate ACT/DVE
                            # (each op ~1.1us; together they keep pace with
                            # warm matmuls at ~1.1us per g8)
                            if (2 * g8 + q // 2) % 2 == 0:
                                nc.scalar.copy(dst, src)
                            else:
                                nc.vector.tensor_copy(dst, src)
                    if g8 == 24:
                        # collective is done by now; emit v0 squash +
                        # G-update here so ACT/DVE interleave it with the
                        # remaining evacs instead of stalling afterwards.
                        # s_full0 content is (k,o); relabel to (o,k).
                        s0v = wk.tile([64, 16, 32], f32, tag="s0v")
                        nc.vector.tensor_copy(
                            s0v[:],
                            s_full0[:].rearrange("p o k -> p (o k)")
                            .rearrange("p (k o) -> p o k", k=K))
                        squash(s0v, vt)
                        update_g(first=True)

            # ---------------- routing passes ----------------
            # NOTE: DVE and GpSimd share SBUF ports -- running Pool TT
            # concurrently with DVE TT slows DVE ~1.5x (measured). All
            # routing TT stays on DVE.
            # Routing-only tiles live in their own pool (rt) so their
            # 39KB/part doesn't shrink phase A's W pools.
            rt_cm = tc.tile_pool(name="rt", bufs=1)
            rt = rt_cm.__enter__()
            DVE_CH = [(0, IC), (IC, IC), (2 * IC, IC), (3 * IC, IC)]
            tmp = rt.tile([128, IC, 16, 32], bf16, tag="tmp")
            tl = rt.tile([128, 16, 32], f32, tag="tl")
            zt = rt.tile([128, 128, 16], bf16, tag="zt")
            rz = rt.tile([128, 128], bf16, tag="rz")
            rzf = rt.tile([128, 128], f32, tag="rzf")

            def b_chunk(eng, tm, i0, ln):
                gb = Grep[:].unsqueeze(1).broadcast_to([128, ln, 16, 32])
                tc_ = tm[:, 0:ln]
                eng.tensor_mul(tc_, u[:, i0:i0 + ln], gb)
                eng.tensor_add(
                    tc_[:, :, 0:8], tc_[:, :, 0:8], tc_[:, :, 8:16])
                eng.tensor_add(
                    tc_[:, :, 0:4], tc_[:, :, 0:4], tc_[:, :, 4:8])
                eng.tensor_add(
                    tc_[:, :, 0:2], tc_[:, :, 0:2], tc_[:, :, 2:4])
                eng.tensor_add(
                    bA[:, i0:i0 + ln, :], tc_[:, :, 0], tc_[:, :, 1])

            def s_chunk(eng, tm, i0, ln, acc, tacc, first):
                cb = bA[:, i0:i0 + ln, :].unsqueeze(2).broadcast_to(
                    [128, ln, 16, 32])
                tc_ = tm[:, 0:ln]
                eng.tensor_mul(tc_, u[:, i0:i0 + ln], cb)
                h = ln // 2
                while h >= 2:
                    eng.tensor_add(
                        tc_[:, 0:h], tc_[:, 0:h], tc_[:, h:2 * h])
                    h //= 2
                if first:
                    eng.tensor_add(acc[:], tc_[:, 0], tc_[:, 1])
                else:
                    eng.tensor_add(tacc[:], tc_[:, 0], tc_[:, 1])
                    eng.tensor_add(acc[:], acc[:], tacc[:])

            for t in (1, 2):
                # b = sum_o u * G; exp per chunk on ACT overlaps the next
                # chunk's DVE work
                for i0, ln in DVE_CH:
                    b_chunk(nc.vector, tmp, i0, ln)
                    nc.scalar.activation(
                        bA[:, i0:i0 + ln, :], bA[:, i0:i0 + ln, :], Act.Exp)
                # dummy Sqrt into the real sq tile (kept alive by its
                # later readers): pulls the sqrt-set table load off the
                # critical path while ACT idles through the s-pass. The
                # exp-set reload then hides under the next b-pass muls.
                nc.scalar.activation(sqs[:], n2s[:], Act.Sqrt)
                # c = softmax_k(b)   (no max subtraction; |b| is small)
                nc.vector.tensor_add(
                    zt[:], bA[:, :, 0:16], bA[:, :, 16:32])
                nc.vector.tensor_add(
                    zt[:, :, 0:8], zt[:, :, 0:8], zt[:, :, 8:16])
                nc.vector.tensor_add(
                    zt[:, :, 0:4], zt[:, :, 0:4], zt[:, :, 4:8])
                nc.vector.tensor_add(
                    zt[:, :, 0:2], zt[:, :, 0:2], zt[:, :, 2:4])
                nc.vector.tensor_add(
                    zt[:, :, 0], zt[:, :, 0], zt[:, :, 1])
                nc.vector.reciprocal(rzf[:], zt[:, :, 0])
                nc.vector.tensor_copy(rz[:], rzf[:])
                rzb = rz[:].unsqueeze(2).broadcast_to([128, 128, 32])
                # c overwrites bA in place
                nc.vector.tensor_mul(bA[:], bA[:], rzb)
                # s(partial) = sum_i c * u
                for n, (i0, ln) in enumerate(DVE_CH):
                    s_chunk(nc.vector, tmp, i0, ln, s_p, tl, n == 0)
                fold_and_allreduce(s_p, s_full)
                squash(s_full, vt)
                if t == 1:
                    update_g(first=False)

            # write out v2 (free layout o*32+k; host reorders)
            nc.sync.dma_start(
                v_d.ap(), vt[:].rearrange("p o k -> p (o k)"))
            rt_cm.__exit__(None, None, None)

    nc.compile()
    return nc


def _prep_inputs(x, W):
    """Host-side shard + relayout + bf16 cast. Per-core input maps."""
    import ml_dtypes

    bf16 = ml_dtypes.bfloat16
    in_maps = []
    for c in range(NCORES):
        lo = c * I_LOC
        xc = x[:, lo:lo + I_LOC, :]                      # [64, 256, 16]
        # xr[i, d, b]
        xr = np.ascontiguousarray(xc.transpose(1, 2, 0))     # [256, 16, 64]
        # x2[32q+16jj+d, g8, 64j'+b] = xr[8g8+2q+jj, d, b] * (jj==j')
        xg = xr.reshape(NG8, 4, 2, D, B)                 # [g8, q, jj, d, b]
        x2 = np.zeros((NG8, 4, 2, D, 2, B), dtype=np.float32)
        x2[:, :, 0, :, 0, :] = xg[:, :, 0]
        x2[:, :, 1, :, 1, :] = xg[:, :, 1]
        # -> [ (q, jj, d) = 128, g8, (j', b) = 128 ]
        x2 = np.ascontiguousarray(
            x2.transpose(1, 2, 3, 0, 4, 5).reshape(128, NG8, 128)
        ).astype(bf16)
        # x8e[16j+d, g8, 64e+b] = xr[8g8+j, d, b] * ((j&1)==e)
        x8g = xr.reshape(NG8, 8, D, B)                   # [g8, j, d, b]
        x8 = np.zeros((NG8, 8, D, 2, B), dtype=np.float32)
        x8[:, 0::2, :, 0, :] = x8g[:, 0::2]
        x8[:, 1::2, :, 1, :] = x8g[:, 1::2]
        x8 = np.ascontiguousarray(
            x8.transpose(1, 2, 0, 3, 4).reshape(128, NG8, 128)).astype(bf16)
        # Wt[g8, 16j+d, 16k+o] = W[lo+8g8+j, k, d, o]
        Wc = W[lo:lo + I_LOC]                            # [256, 32, 16, 16]
        Wt = np.ascontiguousarray(
            Wc.reshape(NG8, 8, K, D, O).transpose(0, 1, 3, 2, 4)
            .reshape(NG8, 128, K * O)).astype(bf16)
        in_maps.append({"x2": x2, "x8e": x8, "Wt": Wt})
    return in_maps


def kernel(**inputs):
    from concourse.bass_utils import run_bass_kernel_spmd

    x = np.ascontiguousarray(inputs["inputs"], dtype=np.float32)
    W = np.ascontiguousarray(inputs["W"], dtype=np.float32)

    if "nc" not in _CACHE:
        _CACHE["nc"] = _build_bass()
    nc = _CACHE["nc"]

    in_maps = _prep_inputs(x, W)
    res = run_bass_kernel_spmd(nc, in_maps, core_ids=list(range(NCORES)))
    v = res.results[0]["v_out"]                          # [64, 512] (o,k)
    return np.ascontiguousarray(
        v.reshape(B, O, K).transpose(0, 2, 1)).astype(np.float32)
